# revision 4
# baseline (speedup 1.0000x reference)
"""Grouped linear (MoE expert GEMM) on 8 NeuronCores, expert-parallel.

Problem: hidden_states [16384, 2048] f32, weight [8, 2048, 2048] f32,
tokens_per_expert [8] = 2048 each (balanced). Output [16384, 2048] f32 with
out[g*2048+t, o] = sum_i x[g*2048+t, i] * weight[g, o, i].

Sharding: expert-parallel — core g gets expert g's weight [2048, 2048] and its
2048 routed tokens; each core runs one 2048x2048x2048 GEMM. No collectives.

Per-core kernel: fp32 data fed to the PE as float32r (4-xbus fp32 streaming,
1 cycle/row at moving-dim >= 256, i.e. full 128x128 MAC rate). X^T is held
fully resident in SBUF (16 tiles of [128, 16, 128]); W^T streams through in
four [128, 16, 512] chunks; PSUM accumulates over the 16 k-chunks of 128.
"""

import numpy as np

G = 8
TPG = 2048  # tokens per expert (= per core)
IN = 2048
OUT = 2048
P = 128
KM = IN // P  # 16 contraction chunks of 128
TT = TPG // P  # 16 token tiles of 128
ON = 4  # number of output-column chunks
OW = OUT // ON  # 512

_nc_cache = {}


def _build_nc():
    import concourse.bacc as bacc
    import concourse.mybir as mybir
    import concourse.tile as tile

    if "nc" in _nc_cache:
        return _nc_cache["nc"]

    f32 = mybir.dt.float32
    f32r = mybir.dt.float32r

    nc = bacc.Bacc(None, target_bir_lowering=False)

    # xt[p, tt, km, tl] = x_core[tt*128+tl, km*128+p]   (X^T, k on partitions)
    xt = nc.dram_tensor("xt", [P, TT, KM, P], f32r, kind="ExternalInput")
    # wt[p, km, o] = w_core[o, km*128+p]                (W^T, k on partitions)
    wt = nc.dram_tensor("wt", [P, KM, OUT], f32r, kind="ExternalInput")
    # out[tt, p, o] = C[tt*128+p, o]
    out = nc.dram_tensor("out", [TT, P, OUT], f32, kind="ExternalOutput")

    with tile.TileContext(nc) as tc:
        with (
            tc.tile_pool(name="xpool", bufs=1) as xpool,
            tc.tile_pool(name="wpool", bufs=2) as wpool,
            tc.tile_pool(name="opool", bufs=4) as opool,
            tc.tile_pool(name="ppool", bufs=6, space="PSUM") as ppool,
            tc.tile_pool(name="warmpool", bufs=1, space="PSUM") as warmpool,
            tc.tile_pool(name="dummypool", bufs=1) as dummypool,
        ):
            # PE warmup: dummy matmuls with no DMA dependency keep the PE busy
            # through the input-fill window so HAM is at full clock (and the
            # pipeline hot) when the first real matmul issues. ~90 x 213ns
            # spans the ~20us fill; HAM re-throttles after ~3.4us idle, so
            # they must run right up to when real work starts.
            dummy = dummypool.tile([P, OW], f32, name="dummy", tag="dummy")
            nc.any.memset(dummy[:], 0.0)
            warm_psum = warmpool.tile([P, OW], f32, name="warm_psum", tag="wp")
            for _ in range(24):
                # Plain fp32 (4 cyc/row): each dummy occupies the PE ~853ns.
                nc.tensor.matmul(
                    out=warm_psum[:],
                    lhsT=dummy[:, 0:P],
                    rhs=dummy[:],
                    start=True,
                    stop=True,
                )
            # Whole X^T resident: 16 x 8KB/partition = 128KB/partition.
            xtiles = []
            for tt in range(TT):
                x_sb = xpool.tile(
                    [P, KM, P], f32r, name=f"x_sb{tt}", tag=f"x{tt}"
                )
                nc.sync.dma_start(out=x_sb[:], in_=xt[:, tt])
                xtiles.append(x_sb)
                if tt == 0:
                    # First W chunk right after the first X tile so compute
                    # can start as soon as ~5MB has landed.
                    w_sb0 = wpool.tile(
                        [P, KM, OW], f32r, name="w_sb0", tag="w"
                    )
                    nc.sync.dma_start(out=w_sb0[:], in_=wt[:, :, 0:OW])

            for oi in range(ON):
                if oi == 0:
                    w_sb = w_sb0
                else:
                    w_sb = wpool.tile(
                        [P, KM, OW], f32r, name=f"w_sb{oi}", tag="w"
                    )
                    nc.sync.dma_start(
                        out=w_sb[:], in_=wt[:, :, oi * OW : (oi + 1) * OW]
                    )
                for tt in range(TT):
                    psum = ppool.tile([P, OW], f32, name="psum", tag="psum")
                    for km in range(KM):
                        nc.tensor.matmul(
                            out=psum[:],
                            lhsT=xtiles[tt][:, km, :],
                            rhs=w_sb[:, km, :],
                            start=(km == 0),
                            stop=(km == KM - 1),
                        )
                    o_sb = opool.tile([P, OW], f32, name="o_sb", tag="o_sb")
                    nc.vector.tensor_copy(out=o_sb[:], in_=psum[:])
                    nc.sync.dma_start(
                        out=out[tt, :, oi * OW : (oi + 1) * OW], in_=o_sb[:]
                    )

    nc.compile()
    _nc_cache["nc"] = nc
    return nc


def _shard_inputs(hidden_states, weight):
    """Host-side reshuffle into the DRAM layouts the kernel expects."""
    x = np.ascontiguousarray(np.asarray(hidden_states, dtype=np.float32))
    w = np.ascontiguousarray(np.asarray(weight, dtype=np.float32))
    in_maps = []
    for g in range(G):
        xg = x[g * TPG : (g + 1) * TPG]  # [2048, 2048]
        # [tt, tl, km, p] -> [p, tt, km, tl]
        xt = np.ascontiguousarray(
            xg.reshape(TT, P, KM, P).transpose(3, 0, 2, 1)
        )
        wg = w[g]  # [out, in]
        # [o, km, p] -> [p, km, o]
        wt = np.ascontiguousarray(
            wg.reshape(OUT, KM, P).transpose(2, 1, 0)
        )
        in_maps.append({"xt": xt, "wt": wt})
    return in_maps


def _run(hidden_states, weight, trace=False, tmpdir=None):
    from concourse.bass_utils import run_bass_kernel_spmd

    nc = _build_nc()
    in_maps = _shard_inputs(hidden_states, weight)
    res = run_bass_kernel_spmd(
        nc, in_maps, core_ids=list(range(G)), trace=trace, tmpdir=tmpdir
    )
    outs = [
        np.asarray(res.results[g]["out"]).reshape(TPG, OUT) for g in range(G)
    ]
    full = np.concatenate(outs, axis=0)
    return full, res


def kernel(hidden_states, weight, tokens_per_expert=None, **_ignored):
    out, _ = _run(hidden_states, weight, trace=False)
    return out
